# revision 4
# baseline (speedup 1.0000x reference)
"""Trainium2 Bass kernel for nn_AutoregressivePolicy_45105746542814.

kernel(**inputs) takes the FULL inputs from setup_inputs() and returns the
full [2, 2048, 32000] fp32 logits.

Sharding (8 NeuronCores): batch (2) x vocab-quarter (4).  Core i = (b, v)
with b = i // 4, v = i % 4 handles token row b and vocab slice
[v*8000, (v+1)*8000).  The upstream (gather / prefix-mean / MLP / LN) is
replicated across the 4 vocab shards of a batch row; fc_out is vocab-sharded.

On-device layout keeps every activation transposed — feature on the SBUF
partition axis, token on the free axis — so each matmul consumes the
previous stage's output directly as its stationary operand and every weight
matrix is used in its natural [in, out] layout:

    embT[p, d, t] = E[tok[t], d*128+p]     (transposing dma_gather)
    hist          = exclusive-prefix-mean  (DVE tensor_tensor_scan, fp32 state)
    curT/henT     = relu(W.T @ x + b)      (TensorE, relu+bias fused in the
                                            ScalarE PSUM-evacuation pass)
    mrgT          = relu(Wm_top.T@cur + Wm_bot.T@hen + bm)
    z             = (mrg - mu) * rsqrt(var + eps)
                    - stats via ones-vector matmuls over the partition axis
                    - ln_w / ln_b are folded into Wo' / bo' on the host
    logits        = z.T @ Wo' + bo'        (Wo' = diag(ln_w) Wo / 10, streamed
                                            from HBM; bias added by VectorE
                                            during PSUM evacuation)

Heavy matmuls run in bf16 with fp32 PSUM accumulation.
"""

from contextlib import ExitStack
from dataclasses import dataclass

import ml_dtypes
import numpy as np

import concourse.bacc as bacc
import concourse.bass as bass
import concourse.mybir as mybir
import concourse.tile as tile
from concourse.bass_utils import run_bass_kernel_spmd

F32 = mybir.dt.float32
BF16 = mybir.dt.bfloat16
I16 = mybir.dt.int16
AF = mybir.ActivationFunctionType
ALU = mybir.AluOpType
BF = ml_dtypes.bfloat16
LN_EPS = 1e-5

N_CORES = 8


@dataclass
class Cfg:
    T: int = 2048      # tokens per core (one batch row)
    D: int = 512       # embed dim
    H: int = 1024      # hidden dim
    VS: int = 8000     # vocab shard per core
    VOCAB: int = 32000
    NF: int = 400      # fc_out free-dim tile (<=512, divides VS)
    NB: int = 512      # MLP token free-dim block

    @property
    def KD(self):
        return self.D // 128

    @property
    def KH(self):
        return self.H // 128

    @property
    def TM(self):
        return self.T // 128

    @property
    def TN(self):
        return self.T // self.NB

    @property
    def NV(self):
        return self.VS // self.NF


def build_nc(cfg: Cfg, num_devices: int = N_CORES, stages: int = 3):
    c = cfg
    nc = bacc.Bacc(
        "TRN2", target_bir_lowering=False, debug=False, num_devices=num_devices
    )

    # ---- DRAM I/O ----
    idx16 = nc.dram_tensor("idx16", [128, c.T // 16], I16, kind="ExternalInput").ap()
    ebf = nc.dram_tensor("ebf", [c.VOCAB, c.D], BF16, kind="ExternalInput").ap()
    recipb = nc.dram_tensor("recipb", [1, c.T], BF16, kind="ExternalInput").ap()
    wc = nc.dram_tensor("wc", [128, c.KD, c.H], BF16, kind="ExternalInput").ap()
    wh = nc.dram_tensor("wh", [128, c.KD, c.H], BF16, kind="ExternalInput").ap()
    wmt = nc.dram_tensor("wmt", [128, c.KH, c.H], BF16, kind="ExternalInput").ap()
    wmb = nc.dram_tensor("wmb", [128, c.KH, c.H], BF16, kind="ExternalInput").ap()
    bcb = nc.dram_tensor("bcb", [128, c.KH], F32, kind="ExternalInput").ap()
    bhb = nc.dram_tensor("bhb", [128, c.KH], F32, kind="ExternalInput").ap()
    bmb = nc.dram_tensor("bmb", [128, c.KH], F32, kind="ExternalInput").ap()
    wot = nc.dram_tensor(
        "wot", [c.NV, 128, c.KH, c.NF], BF16, kind="ExternalInput"
    ).ap()
    bob = nc.dram_tensor("bob", [1, c.VS], BF16, kind="ExternalInput").ap()
    logits = nc.dram_tensor("logits", [c.T, c.VS], F32, kind="ExternalOutput").ap()

    with tile.TileContext(nc) as tc, ExitStack() as es:
        consts = es.enter_context(tc.tile_pool(name="consts", bufs=1))
        acts = es.enter_context(tc.tile_pool(name="acts", bufs=1))
        scanp = es.enter_context(tc.tile_pool(name="scanp", bufs=1))
        blk = es.enter_context(tc.tile_pool(name="blk", bufs=2))
        mm_ps = es.enter_context(tc.tile_pool(name="mm_ps", bufs=2, space="PSUM"))
        st_ps = es.enter_context(tc.tile_pool(name="st_ps", bufs=1, space="PSUM"))
        sqp = es.enter_context(tc.tile_pool(name="sqp", bufs=3))
        smalls = es.enter_context(tc.tile_pool(name="smalls", bufs=2))
        statp = es.enter_context(tc.tile_pool(name="statp", bufs=1))
        wop = es.enter_context(tc.tile_pool(name="wop", bufs=2))
        bobp = es.enter_context(tc.tile_pool(name="bobp", bufs=2))
        outp = es.enter_context(tc.tile_pool(name="outp", bufs=3))
        fo_ps = es.enter_context(tc.tile_pool(name="fo_ps", bufs=4, space="PSUM"))

        # ---- constants / weights into SBUF ----
        idx_sb = consts.tile([128, c.T // 16], I16)
        nc.sync.dma_start(out=idx_sb[:], in_=idx16[:])
        recip_sb = consts.tile([128, c.T], BF16)
        nc.sync.dma_start(out=recip_sb[:], in_=recipb.to_broadcast([128, c.T]))
        ones_col = consts.tile([128, 1], BF16)
        nc.vector.memset(ones_col[:], 1.0)
        ones_row = consts.tile([1, 128], F32)
        nc.vector.memset(ones_row[:], 1.0)
        eps_sb = consts.tile([1, 1], F32)
        nc.vector.memset(eps_sb[:], LN_EPS)

        wc_sb = consts.tile([128, c.KD, c.H], BF16)
        nc.sync.dma_start(out=wc_sb[:], in_=wc[:])
        wh_sb = consts.tile([128, c.KD, c.H], BF16)
        nc.sync.dma_start(out=wh_sb[:], in_=wh[:])
        wmt_sb = consts.tile([128, c.KH, c.H], BF16)
        nc.sync.dma_start(out=wmt_sb[:], in_=wmt[:])
        wmb_sb = consts.tile([128, c.KH, c.H], BF16)
        nc.sync.dma_start(out=wmb_sb[:], in_=wmb[:])
        bc_sb = consts.tile([128, c.KH], F32)
        nc.sync.dma_start(out=bc_sb[:], in_=bcb[:])
        bh_sb = consts.tile([128, c.KH], F32)
        nc.sync.dma_start(out=bh_sb[:], in_=bhb[:])
        bm_sb = consts.tile([128, c.KH], F32)
        nc.sync.dma_start(out=bm_sb[:], in_=bmb[:])

        # ---- gather embeddings, transposed + chunked (dma_gather tops out
        # around 512 indices per call): embT[p, ch, d, i] = E[tok[ch*NB+i], d*128+p]
        CH = c.NB
        NCH = c.T // CH
        embT = acts.tile([128, NCH, c.KD, CH], BF16)
        hist = acts.tile([128, NCH, c.KD, CH], BF16)
        for ch in range(NCH):
            nc.gpsimd.dma_gather(
                embT[:, ch, :, :],
                ebf[:],
                idx_sb[:, ch * (CH // 16) : (ch + 1) * (CH // 16)],
                CH,
                CH,
                c.D,
                transpose=True,
            )

        # ---- prefix-mean (history): hist[t] = (sum_{s<t} emb[s]) / t, hist[0]=emb[0]
        for d in range(c.KD):
            csum = scanp.tile([128, c.T], F32)
            for ch in range(NCH):
                cs = bass.ts(ch, CH)
                nc.vector.tensor_tensor_scan(
                    csum[:, cs],
                    embT[:, ch, d, :],
                    embT[:, ch, d, :],
                    0.0 if ch == 0 else csum[:, ch * CH - 1 : ch * CH],
                    ALU.add,
                    ALU.bypass,
                )
            # hist[t] = csum[t-1] * recip[t] (recip[0]=1), hist[0] = emb[0]
            for ch in range(NCH):
                lo = ch * CH
                if ch == 0:
                    nc.vector.tensor_copy(hist[:, 0, d, 0:1], embT[:, 0, d, 0:1])
                    nc.vector.tensor_mul(
                        hist[:, 0, d, 1:CH], csum[:, 0 : CH - 1], recip_sb[:, 1:CH]
                    )
                else:
                    nc.vector.tensor_mul(
                        hist[:, ch, d, :],
                        csum[:, lo - 1 : lo + CH - 1],
                        recip_sb[:, lo : lo + CH],
                    )

        # ---- per-token-block MLP + LayerNorm -> z (full-T resident) ----
        z = acts.tile([128, c.KH, c.T], BF16)
        for nb in range(c.TN if stages >= 2 else 0):
            ns = bass.ts(nb, c.NB)
            cur = blk.tile([128, c.KH, c.NB], BF16, tag="cur")
            hen = blk.tile([128, c.KH, c.NB], BF16, tag="hen")
            for w_sb, b_sb, src, dst in (
                (wc_sb, bc_sb, embT, cur),
                (wh_sb, bh_sb, hist, hen),
            ):
                for m in range(c.KH):
                    ps = mm_ps.tile([128, c.NB], F32)
                    for k in range(c.KD):
                        nc.tensor.matmul(
                            ps[:],
                            w_sb[:, k, bass.ts(m, 128)],
                            src[:, nb, k, :],
                            start=(k == 0),
                            stop=(k == c.KD - 1),
                        )
                    nc.scalar.activation(
                        dst[:, m, :], ps[:], AF.Relu, bias=b_sb[:, m : m + 1]
                    )

            mrg = blk.tile([128, c.KH, c.NB], BF16, tag="mrg")
            for m in range(c.KH):
                ps = mm_ps.tile([128, c.NB], F32)
                for k in range(c.KH):
                    nc.tensor.matmul(
                        ps[:],
                        wmt_sb[:, k, bass.ts(m, 128)],
                        cur[:, k, :],
                        start=(k == 0),
                        stop=False,
                    )
                for k in range(c.KH):
                    nc.tensor.matmul(
                        ps[:],
                        wmb_sb[:, k, bass.ts(m, 128)],
                        hen[:, k, :],
                        start=False,
                        stop=(k == c.KH - 1),
                    )
                nc.scalar.activation(
                    mrg[:, m, :], ps[:], AF.Relu, bias=bm_sb[:, m : m + 1]
                )

            # LN stats over H (partition axis) via ones-matmuls
            ps_s = st_ps.tile([1, c.NB], F32, tag="s")
            for k in range(c.KH):
                nc.tensor.matmul(
                    ps_s[:],
                    ones_col[:],
                    mrg[:, k, :],
                    start=(k == 0),
                    stop=(k == c.KH - 1),
                )
            ps_s2 = st_ps.tile([1, c.NB], F32, tag="s2")
            for k in range(c.KH):
                sq = sqp.tile([128, c.NB], BF16, tag="sq")
                nc.vector.tensor_mul(sq[:], mrg[:, k, :], mrg[:, k, :])
                nc.tensor.matmul(
                    ps_s2[:],
                    ones_col[:],
                    sq[:],
                    start=(k == 0),
                    stop=(k == c.KH - 1),
                )
            # a = 1/sqrt(var+eps), b = -mu*a   (3 scratch rows)
            b1 = statp.tile([1, c.NB], F32, tag="b1")
            b2 = statp.tile([1, c.NB], F32, tag="b2")
            b3 = statp.tile([1, c.NB], F32, tag="b3")
            nc.vector.tensor_scalar_mul(b1[:], ps_s[:], 1.0 / c.H)  # mu
            nc.vector.tensor_scalar_mul(b2[:], ps_s2[:], 1.0 / c.H)  # E[x^2]
            nc.vector.tensor_mul(b3[:], b1[:], b1[:])  # mu^2
            nc.vector.tensor_sub(b2[:], b2[:], b3[:])  # var
            nc.scalar.activation(b3[:], b2[:], AF.Sqrt, bias=eps_sb[:])  # sd
            nc.vector.reciprocal(b2[:], b3[:])  # a
            nc.vector.tensor_mul(b3[:], b1[:], b2[:])
            nc.vector.tensor_scalar_mul(b3[:], b3[:], -1.0)  # b

            # broadcast a/b across partitions via K=1 fp32 matmuls
            ab_b = smalls.tile([128, c.NB], BF16, tag="ab")
            bb_b = smalls.tile([128, c.NB], BF16, tag="bb")
            for src1, dst1 in ((b2, ab_b), (b3, bb_b)):
                ps = mm_ps.tile([128, c.NB], F32)
                nc.tensor.matmul(ps[:], ones_row[:], src1[:], start=True, stop=True)
                nc.scalar.copy(dst1[:], ps[:])

            # z = mrg * a + b
            for m in range(c.KH):
                tmp = sqp.tile([128, c.NB], BF16, tag="tmpn")
                nc.vector.tensor_mul(tmp[:], mrg[:, m, :], ab_b[:])
                nc.vector.tensor_add(z[:, m, ns], tmp[:], bb_b[:])

        if stages < 2:
            # debug: dump first d-tile of hist (chunk 0)
            nc.gpsimd.dma_start(
                out=logits[0:128, 0 : c.NB], in_=hist[:, 0, 0, :]
            )
        # ---- fc_out: logits[t, v] = z.T @ Wo' + bo' ----
        for n in range(c.NV if stages >= 3 else 0):
            wo_sb = wop.tile([128, c.KH, c.NF], BF16)
            if n % 2 == 0 or n < 7:
                nc.sync.dma_start(out=wo_sb[:], in_=wot[n])
            else:
                nc.gpsimd.dma_start(out=wo_sb[:], in_=wot[n])
            GB = min(8, c.NV)
            if n % GB == 0:
                bob_g = bobp.tile([128, GB * c.NF], BF16)
                nc.scalar.dma_start(
                    out=bob_g[:],
                    in_=bob[:, n * c.NF : (n + GB) * c.NF].to_broadcast(
                        [128, GB * c.NF]
                    ),
                )
            bob_n = bob_g[:, (n % GB) * c.NF : (n % GB + 1) * c.NF]
            for m in range(c.TM):
                ps = fo_ps.tile([128, c.NF], F32)
                for k in range(c.KH):
                    nc.tensor.matmul(
                        ps[:],
                        z[:, k, bass.ts(m, 128)],
                        wo_sb[:, k, :],
                        start=(k == 0),
                        stop=(k == c.KH - 1),
                    )
                ot = outp.tile([128, c.NF], F32)
                nc.vector.tensor_add(ot[:], ps[:], bob_n)
                nc.scalar.dma_start(
                    out=logits[bass.ts(m, 128), bass.ts(n, c.NF)], in_=ot[:]
                )

    nc.compile()
    return nc


def prep_core_inputs(cfg, tokens_row, ebf16, Wc, bc, Wh, bh, Wm, bm, Wo, bo,
                     ln_w, ln_b, v0):
    """Build the in_map for one core: token row `tokens_row` (length T),
    vocab slice [v0, v0+VS)."""
    c = cfg
    T, VS, NF, NV, KH = c.T, c.VS, c.NF, c.NV, c.KH

    tok = np.asarray(tokens_row).astype(np.int64).ravel()
    assert tok.shape[0] == T
    # dma_gather unwraps indices as g = s*16 + p from a [16, S] view,
    # replicated across the 8 Q7 cores -> [128, S].
    idx16 = np.tile(
        np.ascontiguousarray(tok.astype(np.int16).reshape(T // 16, 16).T), (8, 1)
    )

    recip = np.ones((1, T), np.float32)
    recip[0, 1:] = 1.0 / np.arange(1, T, dtype=np.float32)

    Wo_s = (np.asarray(ln_w, np.float64)[:, None] * np.asarray(Wo, np.float64)
            / 10.0)[:, v0 : v0 + VS]
    bo_s = (
        (np.asarray(ln_b, np.float64) @ np.asarray(Wo, np.float64)
         + np.asarray(bo, np.float64)) / 10.0
    )[v0 : v0 + VS]
    wot = np.ascontiguousarray(
        Wo_s.reshape(KH, 128, NV, NF).transpose(2, 1, 0, 3).astype(BF)
    )

    def btile(b):
        return np.ascontiguousarray(np.asarray(b, np.float32).reshape(KH, 128).T)

    return {
        "idx16": idx16,
        "ebf": ebf16,
        "recipb": recip.astype(BF),
        "wc": np.asarray(Wc).astype(BF),
        "wh": np.asarray(Wh).astype(BF),
        "wmt": np.asarray(Wm)[: c.H].astype(BF),
        "wmb": np.asarray(Wm)[c.H :].astype(BF),
        "bcb": btile(bc),
        "bhb": btile(bh),
        "bmb": btile(bm),
        "wot": wot,
        "bob": bo_s.reshape(1, VS).astype(BF),
    }




@dataclass
class CfgC:
    T: int = 2048      # full sequence length
    TL: int = 512      # local tokens per core
    D: int = 512
    H: int = 1024
    V: int = 32000     # full vocab (per core)
    NF: int = 500      # fc_out free-dim tile

    @property
    def KD(self):
        return self.D // 128

    @property
    def KH(self):
        return self.H // 128

    @property
    def NC(self):
        return self.T // 128   # 128-token chunks in sequence

    @property
    def LC(self):
        return self.TL // 128  # local chunks

    @property
    def TM(self):
        return self.TL // 128  # fc_out token tiles

    @property
    def NV(self):
        return self.V // self.NF


def build_nc_c(cfg: CfgC, num_devices: int = 8):
    c = cfg
    GCH = min(512, c.T)          # idxs per dma_gather call
    NG = c.T // GCH              # gather calls for full sequence
    nc = bacc.Bacc(
        "TRN2", target_bir_lowering=False, debug=False, num_devices=num_devices
    )

    # ---- DRAM I/O (per core) ----
    idxa = nc.dram_tensor("idxa", [128, c.T // 16], I16, kind="ExternalInput").ap()
    idxb = nc.dram_tensor("idxb", [128, c.TL // 16], I16, kind="ExternalInput").ap()
    ebf = nc.dram_tensor("ebf", [c.V, c.D], BF16, kind="ExternalInput").ap()
    oneh = nc.dram_tensor("oneh", [128, c.NC, c.NC], BF16, kind="ExternalInput").ap()
    umat = nc.dram_tensor("umat", [c.NC, 128], BF16, kind="ExternalInput").ap()
    lsc = nc.dram_tensor("lsc", [128, c.LC, 128], BF16, kind="ExternalInput").ap()
    rro = nc.dram_tensor("rro", [128, 128], BF16, kind="ExternalInput").ap()
    wc = nc.dram_tensor("wc", [128, c.KD, c.H], BF16, kind="ExternalInput").ap()
    wh = nc.dram_tensor("wh", [128, c.KD, c.H], BF16, kind="ExternalInput").ap()
    wmt = nc.dram_tensor("wmt", [128, c.KH, c.H], BF16, kind="ExternalInput").ap()
    wmb = nc.dram_tensor("wmb", [128, c.KH, c.H], BF16, kind="ExternalInput").ap()
    bcb = nc.dram_tensor("bcb", [128, c.KH], F32, kind="ExternalInput").ap()
    bhb = nc.dram_tensor("bhb", [128, c.KH], F32, kind="ExternalInput").ap()
    bmb = nc.dram_tensor("bmb", [128, c.KH], F32, kind="ExternalInput").ap()
    wot = nc.dram_tensor(
        "wot", [c.NV, 128, c.KH, c.NF], BF16, kind="ExternalInput"
    ).ap()
    bob = nc.dram_tensor("bob", [1, c.V], BF16, kind="ExternalInput").ap()
    logits = nc.dram_tensor("logits", [c.TL, c.V], F32, kind="ExternalOutput").ap()

    with tile.TileContext(nc) as tc, ExitStack() as es:
        consts = es.enter_context(tc.tile_pool(name="consts", bufs=1))
        acts = es.enter_context(tc.tile_pool(name="acts", bufs=1))
        mm_ps = es.enter_context(tc.tile_pool(name="mm_ps", bufs=2, space="PSUM"))
        st_ps = es.enter_context(tc.tile_pool(name="st_ps", bufs=1, space="PSUM"))
        sqp = es.enter_context(tc.tile_pool(name="sqp", bufs=3))
        smalls = es.enter_context(tc.tile_pool(name="smalls", bufs=2))
        statp = es.enter_context(tc.tile_pool(name="statp", bufs=1))
        wop = es.enter_context(tc.tile_pool(name="wop", bufs=6))
        bobp = es.enter_context(tc.tile_pool(name="bobp", bufs=3))
        outp = es.enter_context(tc.tile_pool(name="outp", bufs=4))
        fo_ps = es.enter_context(tc.tile_pool(name="fo_ps", bufs=4, space="PSUM"))

        # ---- constants / weights ----
        idxa_sb = consts.tile([128, c.T // 16], I16)
        nc.sync.dma_start(out=idxa_sb[:], in_=idxa[:])
        idxb_sb = consts.tile([128, c.TL // 16], I16)
        nc.sync.dma_start(out=idxb_sb[:], in_=idxb[:])
        oneh_sb = consts.tile([128, c.NC, c.NC], BF16)
        nc.sync.dma_start(out=oneh_sb[:], in_=oneh[:])
        umat_sb = consts.tile([c.NC, 128], BF16)
        nc.sync.dma_start(out=umat_sb[:], in_=umat[:])
        lsc_sb = consts.tile([128, c.LC, 128], BF16)
        nc.sync.dma_start(out=lsc_sb[:], in_=lsc[:])
        rro_sb = consts.tile([128, 128], BF16)
        nc.sync.dma_start(out=rro_sb[:], in_=rro[:])
        ones_col = consts.tile([128, 1], BF16)
        nc.vector.memset(ones_col[:], 1.0)
        ones_row = consts.tile([1, 128], F32)
        nc.vector.memset(ones_row[:], 1.0)
        eps_sb = consts.tile([1, 1], F32)
        nc.vector.memset(eps_sb[:], LN_EPS)

        wc_sb = consts.tile([128, c.KD, c.H], BF16)
        nc.sync.dma_start(out=wc_sb[:], in_=wc[:])
        wh_sb = consts.tile([128, c.KD, c.H], BF16)
        nc.sync.dma_start(out=wh_sb[:], in_=wh[:])
        wmt_sb = consts.tile([128, c.KH, c.H], BF16)
        nc.sync.dma_start(out=wmt_sb[:], in_=wmt[:])
        wmb_sb = consts.tile([128, c.KH, c.H], BF16)
        nc.sync.dma_start(out=wmb_sb[:], in_=wmb[:])
        bc_sb = consts.tile([128, c.KH], F32)
        nc.sync.dma_start(out=bc_sb[:], in_=bcb[:])
        bh_sb = consts.tile([128, c.KH], F32)
        nc.sync.dma_start(out=bh_sb[:], in_=bhb[:])
        bm_sb = consts.tile([128, c.KH], F32)
        nc.sync.dma_start(out=bm_sb[:], in_=bmb[:])

        # ---- gathers ----
        # embA[p, sc, d] = emb_rot[sc*128+p, d]  (rotated order, row layout)
        embA = acts.tile([128, c.NC, c.D], BF16)
        for g in range(NG):
            nc.gpsimd.dma_gather(
                embA[:, g * (GCH // 128) : (g + 1) * (GCH // 128), :],
                ebf[:],
                idxa_sb[:, g * (GCH // 16) : (g + 1) * (GCH // 16)],
                GCH,
                GCH,
                c.D,
            )
        # embB[p, k, t] = E[loc_tok[t], k*128+p]  (bf16, transposed gather)
        embB = acts.tile([128, c.KD, c.TL], BF16)
        nc.gpsimd.dma_gather(
            embB[:], ebf[:], idxb_sb[:], c.TL, c.TL, c.D, transpose=True
        )

        # ---- chunk sums S[sc, d] then carries = U.T @ S ----
        ps_S = st_ps.tile([c.NC, c.D], F32, tag="s")
        for sc in range(c.NC):
            nc.tensor.matmul(
                ps_S[:],
                oneh_sb[:, sc, :],
                embA[:, sc, :],
                start=(sc == 0),
                stop=(sc == c.NC - 1),
            )
        S_sb = smalls.tile([c.NC, c.D], BF16, tag="S")
        nc.vector.tensor_copy(S_sb[:], ps_S[:])
        # carries spread directly to partitions 32*j via the U-selector matmul
        ps_C = st_ps.tile([128, c.D], F32, tag="s2")
        nc.tensor.matmul(ps_C[:], umat_sb[:], S_sb[:], start=True, stop=True)
        carry_sb = consts.tile([128, c.D], BF16)
        nc.scalar.copy(carry_sb[:], ps_C[:])

        # ---- local prefix means, transposed: histT[p, m, j*128+t] ----
        histT = acts.tile([128, c.KD, c.TL], BF16)
        for j in range(c.LC):
            for m in range(c.KD):
                ph = mm_ps.tile([128, 128], F32, tag="ps")
                nc.tensor.matmul(
                    ph[:],
                    embA[:, j, bass.ts(m, 128)],
                    lsc_sb[:, j, :],
                    start=True,
                    stop=False,
                )
                nc.tensor.matmul(
                    ph[:],
                    carry_sb[32 * j : 32 * j + 1, bass.ts(m, 128)],
                    rro_sb[32 * j : 32 * j + 1, :],
                    start=False,
                    stop=True,
                    tile_position=(32 * j, 0),
                )
                nc.scalar.copy(histT[:, m, bass.ts(j, 128)], ph[:])

        # ---- MLP (single 512-token block) ----
        cur = acts.tile([128, c.KH, c.TL], BF16)
        hen = acts.tile([128, c.KH, c.TL], BF16)
        for w_sb, b_sb, src, dst in (
            (wc_sb, bc_sb, embB, cur),
            (wh_sb, bh_sb, histT, hen),
        ):
            for m in range(c.KH):
                ps = mm_ps.tile([128, c.TL], F32, tag="ps")
                for k in range(c.KD):
                    nc.tensor.matmul(
                        ps[:],
                        w_sb[:, k, bass.ts(m, 128)],
                        src[:, k, :],
                        start=(k == 0),
                        stop=(k == c.KD - 1),
                    )
                nc.scalar.activation(
                    dst[:, m, :], ps[:], AF.Relu, bias=b_sb[:, m : m + 1]
                )

        mrg = acts.tile([128, c.KH, c.TL], BF16)
        for m in range(c.KH):
            ps = mm_ps.tile([128, c.TL], F32, tag="ps")
            for k in range(c.KH):
                nc.tensor.matmul(
                    ps[:],
                    wmt_sb[:, k, bass.ts(m, 128)],
                    cur[:, k, :],
                    start=(k == 0),
                    stop=False,
                )
            for k in range(c.KH):
                nc.tensor.matmul(
                    ps[:],
                    wmb_sb[:, k, bass.ts(m, 128)],
                    hen[:, k, :],
                    start=False,
                    stop=(k == c.KH - 1),
                )
            nc.scalar.activation(
                mrg[:, m, :], ps[:], AF.Relu, bias=bm_sb[:, m : m + 1]
            )

        # ---- LayerNorm ----
        ps_s = st_ps.tile([1, c.TL], F32, tag="s")
        for k in range(c.KH):
            nc.tensor.matmul(
                ps_s[:], ones_col[:], mrg[:, k, :],
                start=(k == 0), stop=(k == c.KH - 1),
            )
        ps_s2 = st_ps.tile([1, c.TL], F32, tag="s2")
        for k in range(c.KH):
            sq = sqp.tile([128, c.TL], BF16, tag="sq")
            nc.vector.tensor_mul(sq[:], mrg[:, k, :], mrg[:, k, :])
            nc.tensor.matmul(
                ps_s2[:], ones_col[:], sq[:],
                start=(k == 0), stop=(k == c.KH - 1),
            )
        b1 = statp.tile([1, c.TL], F32, tag="b1")
        b2 = statp.tile([1, c.TL], F32, tag="b2")
        b3 = statp.tile([1, c.TL], F32, tag="b3")
        nc.vector.tensor_scalar_mul(b1[:], ps_s[:], 1.0 / c.H)
        nc.vector.tensor_scalar_mul(b2[:], ps_s2[:], 1.0 / c.H)
        nc.vector.tensor_mul(b3[:], b1[:], b1[:])
        nc.vector.tensor_sub(b2[:], b2[:], b3[:])
        nc.scalar.activation(b3[:], b2[:], AF.Sqrt, bias=eps_sb[:])
        nc.vector.reciprocal(b2[:], b3[:])
        nc.vector.tensor_mul(b3[:], b1[:], b2[:])
        nc.vector.tensor_scalar_mul(b3[:], b3[:], -1.0)

        ab_b = smalls.tile([128, c.TL], BF16, tag="ab")
        bb_b = smalls.tile([128, c.TL], BF16, tag="bb")
        for src1, dst1 in ((b2, ab_b), (b3, bb_b)):
            ps = mm_ps.tile([128, c.TL], F32, tag="ps")
            nc.tensor.matmul(ps[:], ones_row[:], src1[:], start=True, stop=True)
            nc.scalar.copy(dst1[:], ps[:])

        z = acts.tile([128, c.KH, c.TL], BF16)
        for m in range(c.KH):
            tmp = sqp.tile([128, c.TL], BF16, tag="tmpn")
            nc.vector.tensor_mul(tmp[:], mrg[:, m, :], ab_b[:])
            nc.vector.tensor_add(z[:, m, :], tmp[:], bb_b[:])

        # ---- fc_out over full vocab ----
        for n in range(c.NV):
            wo_sb = wop.tile([128, c.KH, c.NF], BF16)
            if n % 2 == 0 or n < 7:
                nc.sync.dma_start(out=wo_sb[:], in_=wot[n])
            else:
                nc.gpsimd.dma_start(out=wo_sb[:], in_=wot[n])
            GB = min(8, c.NV)
            if n % GB == 0:
                bob_g = bobp.tile([128, GB * c.NF], BF16)
                nc.scalar.dma_start(
                    out=bob_g[:],
                    in_=bob[:, n * c.NF : (n + GB) * c.NF].to_broadcast(
                        [128, GB * c.NF]
                    ),
                )
            bob_n = bob_g[:, (n % GB) * c.NF : (n % GB + 1) * c.NF]
            for m in range(c.TM):
                ps = fo_ps.tile([128, c.NF], F32)
                for k in range(c.KH):
                    nc.tensor.matmul(
                        ps[:],
                        z[:, k, bass.ts(m, 128)],
                        wo_sb[:, k, :],
                        start=(k == 0),
                        stop=(k == c.KH - 1),
                    )
                ot = outp.tile([128, c.NF], F32)
                nc.vector.tensor_add(ot[:], ps[:], bob_n)
                nc.scalar.dma_start(
                    out=logits[bass.ts(m, 128), bass.ts(n, c.NF)], in_=ot[:]
                )

    nc.compile()
    return nc


def prep_core_inputs_c(cfg, tokens_row, e32c, ebf16, Wo_s, bo_s, Wc, bc, Wh, bh,
                       Wm, bm, q):
    """Per-core in_map for token quarter q of one batch row."""
    c = cfg
    T, TL, NC, LC, KH, NF, NV = c.T, c.TL, c.NC, c.LC, c.KH, c.NF, c.NV

    tok = np.asarray(tokens_row).astype(np.int64).ravel()
    assert tok.shape[0] == T

    def wrap(ix):
        n = ix.shape[0]
        return np.tile(
            np.ascontiguousarray(ix.astype(np.int16).reshape(n // 16, 16).T), (8, 1)
        )

    # rotation: local chunks first, then the rest in order
    loc = list(range(LC * q, LC * (q + 1)))
    order = loc + [g for g in range(NC) if g not in loc]
    tokrot = np.concatenate([tok[g * 128 : (g + 1) * 128] for g in order])
    idxa = wrap(tokrot)
    idxb = wrap(tok[TL * q : TL * (q + 1)])

    oneh = np.zeros((128, NC, NC), np.float32)
    for sc in range(NC):
        oneh[:, sc, sc] = 1.0
    umat = np.zeros((NC, 128), np.float32)
    for j in range(LC):
        for jp in range(NC):
            if order[jp] < order[j]:
                umat[jp, 32 * j] = 1.0

    lsc = np.zeros((128, LC, 128), np.float32)
    rro = np.zeros((128, 128), np.float32)
    for j in range(LC):
        g = order[j]
        gt = g * 128 + np.arange(128)
        col = np.where(gt > 0, 1.0 / np.maximum(gt, 1), 1.0).astype(np.float32)
        s_idx = np.arange(128)[:, None]
        t_idx = np.arange(128)[None, :]
        lsc[:, j, :] = np.where(s_idx < t_idx, col[None, :], 0.0)
        if g == 0:
            lsc[0, j, 0] = 1.0
        rro[32 * j, :] = col

    def btile(b):
        return np.ascontiguousarray(np.asarray(b, np.float32).reshape(KH, 128).T)

    wot = np.ascontiguousarray(
        Wo_s.reshape(KH, 128, NV, NF).transpose(2, 1, 0, 3).astype(BF)
    )

    def ktile(w, kt):
        return np.ascontiguousarray(
            np.asarray(w, np.float32).reshape(kt, 128, c.H).transpose(1, 0, 2)
        ).astype(BF)

    return {
        "idxa": idxa,
        "idxb": idxb,
        "ebf": ebf16,
        "oneh": oneh.astype(BF),
        "umat": umat.astype(BF),
        "lsc": lsc.astype(BF),
        "rro": rro.astype(BF),
        "wc": ktile(Wc, c.KD),
        "wh": ktile(Wh, c.KD),
        "wmt": ktile(np.asarray(Wm)[: c.H], c.KH),
        "wmb": ktile(np.asarray(Wm)[c.H :], c.KH),
        "bcb": btile(bc),
        "bhb": btile(bh),
        "bmb": btile(bm),
        "wot": wot,
        "bob": bo_s.reshape(1, c.V).astype(BF),
    }


def build_nc_d(cfg: CfgC, num_devices: int = 8):
    """Tuned rev of build_nc_c:

    - logits emitted bf16 (halves the output DMA; host upcasts)
    - PE emission order overlaps the cur-path MLP with the embA gathers
      (cur first, then chunk-sums/carry/prefix, hen, mrg)
    - embA split into one tile per dma_gather call so chunk-sums can start
      as soon as their chunk lands
    - const DMAs ordered by first use
    - PSUM pools scoped: prologue {mm_ps, st_ps}, then fc_out gets all 8
      banks (fo_ps bufs=8)
    - fc_out biases prefetched 3 groups ahead (GB=4, bufs=4)
    - wo prefetch depth 7
    """
    c = cfg
    GCH = min(512, c.T)          # idxs per dma_gather call
    NG = c.T // GCH              # gather calls for full sequence
    SCG = GCH // 128             # 128-token chunks per gather call
    nc = bacc.Bacc(
        "TRN2", target_bir_lowering=False, debug=False, num_devices=num_devices
    )

    # ---- DRAM I/O (per core) ----
    idxa = nc.dram_tensor("idxa", [128, c.T // 16], I16, kind="ExternalInput").ap()
    idxb = nc.dram_tensor("idxb", [128, c.TL // 16], I16, kind="ExternalInput").ap()
    ebf = nc.dram_tensor("ebf", [c.V, c.D], BF16, kind="ExternalInput").ap()
    oneh = nc.dram_tensor("oneh", [128, c.NC, c.NC], BF16, kind="ExternalInput").ap()
    umat = nc.dram_tensor("umat", [c.NC, 128], BF16, kind="ExternalInput").ap()
    lsc = nc.dram_tensor("lsc", [128, c.LC, 128], BF16, kind="ExternalInput").ap()
    rro = nc.dram_tensor("rro", [128, 128], BF16, kind="ExternalInput").ap()
    wc = nc.dram_tensor("wc", [128, c.KD, c.H], BF16, kind="ExternalInput").ap()
    wh = nc.dram_tensor("wh", [128, c.KD, c.H], BF16, kind="ExternalInput").ap()
    wmt = nc.dram_tensor("wmt", [128, c.KH, c.H], BF16, kind="ExternalInput").ap()
    wmb = nc.dram_tensor("wmb", [128, c.KH, c.H], BF16, kind="ExternalInput").ap()
    bcb = nc.dram_tensor("bcb", [128, c.KH], F32, kind="ExternalInput").ap()
    bhb = nc.dram_tensor("bhb", [128, c.KH], F32, kind="ExternalInput").ap()
    bmb = nc.dram_tensor("bmb", [128, c.KH], F32, kind="ExternalInput").ap()
    wot = nc.dram_tensor(
        "wot", [c.NV, 128, c.KH, c.NF], BF16, kind="ExternalInput"
    ).ap()
    bob = nc.dram_tensor("bob", [1, c.V], BF16, kind="ExternalInput").ap()
    logits = nc.dram_tensor("logits", [c.TL, c.V], BF16, kind="ExternalOutput").ap()

    with tile.TileContext(nc) as tc, ExitStack() as es:
        consts = es.enter_context(tc.tile_pool(name="consts", bufs=1))
        acts = es.enter_context(tc.tile_pool(name="acts", bufs=1))
        sqp = es.enter_context(tc.tile_pool(name="sqp", bufs=2))
        smalls = es.enter_context(tc.tile_pool(name="smalls", bufs=2))
        statp = es.enter_context(tc.tile_pool(name="statp", bufs=1))
        wop = es.enter_context(tc.tile_pool(name="wop", bufs=7))
        bobp = es.enter_context(tc.tile_pool(name="bobp", bufs=4))
        outp = es.enter_context(tc.tile_pool(name="outp", bufs=4))

        # ---- constants, ordered by first use on the sync DMA queue ----
        idxb_sb = consts.tile([128, c.TL // 16], I16)
        nc.sync.dma_start(out=idxb_sb[:], in_=idxb[:])
        idxa_sb = consts.tile([128, c.T // 16], I16)
        nc.sync.dma_start(out=idxa_sb[:], in_=idxa[:])
        bc_sb = consts.tile([128, c.KH], F32)
        nc.sync.dma_start(out=bc_sb[:], in_=bcb[:])
        wc_sb = consts.tile([128, c.KD, c.H], BF16)
        nc.sync.dma_start(out=wc_sb[:], in_=wc[:])
        oneh_sb = consts.tile([128, c.NC, c.NC], BF16)
        nc.sync.dma_start(out=oneh_sb[:], in_=oneh[:])
        umat_sb = consts.tile([c.NC, 128], BF16)
        nc.sync.dma_start(out=umat_sb[:], in_=umat[:])
        lsc_sb = consts.tile([128, c.LC, 128], BF16)
        nc.sync.dma_start(out=lsc_sb[:], in_=lsc[:])
        rro_sb = consts.tile([128, 128], BF16)
        nc.sync.dma_start(out=rro_sb[:], in_=rro[:])
        bh_sb = consts.tile([128, c.KH], F32)
        nc.sync.dma_start(out=bh_sb[:], in_=bhb[:])
        wh_sb = consts.tile([128, c.KD, c.H], BF16)
        nc.sync.dma_start(out=wh_sb[:], in_=wh[:])
        bm_sb = consts.tile([128, c.KH], F32)
        nc.sync.dma_start(out=bm_sb[:], in_=bmb[:])
        wmt_sb = consts.tile([128, c.KH, c.H], BF16)
        nc.sync.dma_start(out=wmt_sb[:], in_=wmt[:])
        wmb_sb = consts.tile([128, c.KH, c.H], BF16)
        nc.sync.dma_start(out=wmb_sb[:], in_=wmb[:])
        ones_col = consts.tile([128, 1], BF16)
        nc.vector.memset(ones_col[:], 1.0)
        ones_row = consts.tile([1, 128], F32)
        nc.vector.memset(ones_row[:], 1.0)
        eps_sb = consts.tile([1, 1], F32)
        nc.vector.memset(eps_sb[:], LN_EPS)

        # ---- gathers: embB (cur path) first, then embA in per-call tiles ----
        embB = acts.tile([128, c.KD, c.TL], BF16)
        nc.gpsimd.dma_gather(
            embB[:], ebf[:], idxb_sb[:], c.TL, c.TL, c.D, transpose=True
        )
        embAg = []
        for g in range(NG):
            t = acts.tile([128, SCG, c.D], BF16, name=f"embA{g}")
            nc.gpsimd.dma_gather(
                t[:],
                ebf[:],
                idxa_sb[:, g * (GCH // 16) : (g + 1) * (GCH // 16)],
                GCH,
                GCH,
                c.D,
            )
            embAg.append(t)

        cur = acts.tile([128, c.KH, c.TL], BF16)
        hen = acts.tile([128, c.KH, c.TL], BF16)
        histT = acts.tile([128, c.KD, c.TL], BF16)
        mrg = acts.tile([128, c.KH, c.TL], BF16)
        z = acts.tile([128, c.KH, c.TL], BF16)

        with ExitStack() as pes:
            mm_ps = pes.enter_context(
                tc.tile_pool(name="mm_ps", bufs=2, space="PSUM")
            )
            st_ps = pes.enter_context(
                tc.tile_pool(name="st_ps", bufs=1, space="PSUM")
            )

            # [1] cur-path MLP (only needs embB + wc; runs during embA gathers)
            for m in range(c.KH):
                ps = mm_ps.tile([128, c.TL], F32, tag="ps")
                for k in range(c.KD):
                    nc.tensor.matmul(
                        ps[:],
                        wc_sb[:, k, bass.ts(m, 128)],
                        embB[:, k, :],
                        start=(k == 0),
                        stop=(k == c.KD - 1),
                    )
                nc.scalar.activation(
                    cur[:, m, :], ps[:], AF.Relu, bias=bc_sb[:, m : m + 1]
                )

            # [2] chunk sums S[sc, d] as embA chunks land
            ps_S = st_ps.tile([c.NC, c.D], F32, tag="s")
            for sc in range(c.NC):
                nc.tensor.matmul(
                    ps_S[:],
                    oneh_sb[:, sc, :],
                    embAg[sc // SCG][:, sc % SCG, :],
                    start=(sc == 0),
                    stop=(sc == c.NC - 1),
                )
            S_sb = smalls.tile([c.NC, c.D], BF16, tag="S")
            nc.vector.tensor_copy(S_sb[:], ps_S[:])
            # carries spread directly to partitions 32*j via the U-selector
            ps_C = st_ps.tile([128, c.D], F32, tag="s2")
            nc.tensor.matmul(ps_C[:], umat_sb[:], S_sb[:], start=True, stop=True)
            carry_sb = consts.tile([128, c.D], BF16)
            nc.scalar.copy(carry_sb[:], ps_C[:])

            # [3] local prefix means, transposed: histT[p, m, j*128+t]
            for j in range(c.LC):
                for m in range(c.KD):
                    ph = mm_ps.tile([128, 128], F32, tag="ph")
                    nc.tensor.matmul(
                        ph[:],
                        embAg[j // SCG][:, j % SCG, bass.ts(m, 128)],
                        lsc_sb[:, j, :],
                        start=True,
                        stop=False,
                    )
                    nc.tensor.matmul(
                        ph[:],
                        carry_sb[32 * j : 32 * j + 1, bass.ts(m, 128)],
                        rro_sb[32 * j : 32 * j + 1, :],
                        start=False,
                        stop=True,
                        tile_position=(32 * j, 0),
                    )
                    nc.scalar.copy(histT[:, m, bass.ts(j, 128)], ph[:])

            # [4] hist-path MLP
            for m in range(c.KH):
                ps = mm_ps.tile([128, c.TL], F32, tag="ps")
                for k in range(c.KD):
                    nc.tensor.matmul(
                        ps[:],
                        wh_sb[:, k, bass.ts(m, 128)],
                        histT[:, k, :],
                        start=(k == 0),
                        stop=(k == c.KD - 1),
                    )
                nc.scalar.activation(
                    hen[:, m, :], ps[:], AF.Relu, bias=bh_sb[:, m : m + 1]
                )

            # [5] merge MLP
            for m in range(c.KH):
                ps = mm_ps.tile([128, c.TL], F32, tag="ps")
                for k in range(c.KH):
                    nc.tensor.matmul(
                        ps[:],
                        wmt_sb[:, k, bass.ts(m, 128)],
                        cur[:, k, :],
                        start=(k == 0),
                        stop=False,
                    )
                for k in range(c.KH):
                    nc.tensor.matmul(
                        ps[:],
                        wmb_sb[:, k, bass.ts(m, 128)],
                        hen[:, k, :],
                        start=False,
                        stop=(k == c.KH - 1),
                    )
                nc.scalar.activation(
                    mrg[:, m, :], ps[:], AF.Relu, bias=bm_sb[:, m : m + 1]
                )

            # [6] LayerNorm
            ps_s = st_ps.tile([1, c.TL], F32, tag="s")
            for k in range(c.KH):
                nc.tensor.matmul(
                    ps_s[:], ones_col[:], mrg[:, k, :],
                    start=(k == 0), stop=(k == c.KH - 1),
                )
            ps_s2 = st_ps.tile([1, c.TL], F32, tag="s2")
            for k in range(c.KH):
                sq = sqp.tile([128, c.TL], BF16, tag="sq")
                nc.vector.tensor_mul(sq[:], mrg[:, k, :], mrg[:, k, :])
                nc.tensor.matmul(
                    ps_s2[:], ones_col[:], sq[:],
                    start=(k == 0), stop=(k == c.KH - 1),
                )
            b1 = statp.tile([1, c.TL], F32, tag="b1")
            b2 = statp.tile([1, c.TL], F32, tag="b2")
            b3 = statp.tile([1, c.TL], F32, tag="b3")
            nc.vector.tensor_scalar_mul(b1[:], ps_s[:], 1.0 / c.H)
            nc.vector.tensor_scalar_mul(b2[:], ps_s2[:], 1.0 / c.H)
            nc.vector.tensor_mul(b3[:], b1[:], b1[:])
            nc.vector.tensor_sub(b2[:], b2[:], b3[:])
            nc.scalar.activation(b3[:], b2[:], AF.Sqrt, bias=eps_sb[:])
            nc.vector.reciprocal(b2[:], b3[:])
            nc.vector.tensor_mul(b3[:], b1[:], b2[:])
            nc.vector.tensor_scalar_mul(b3[:], b3[:], -1.0)

            ab_b = smalls.tile([128, c.TL], BF16, tag="ab")
            bb_b = smalls.tile([128, c.TL], BF16, tag="bb")
            for src1, dst1 in ((b2, ab_b), (b3, bb_b)):
                ps = mm_ps.tile([128, c.TL], F32, tag="ps")
                nc.tensor.matmul(ps[:], ones_row[:], src1[:], start=True, stop=True)
                nc.scalar.copy(dst1[:], ps[:])

            for m in range(c.KH):
                tmp = sqp.tile([128, c.TL], BF16, tag="tmpn")
                nc.vector.tensor_mul(tmp[:], mrg[:, m, :], ab_b[:])
                nc.vector.tensor_add(z[:, m, :], tmp[:], bb_b[:])

        # ---- fc_out over full vocab: all 8 PSUM banks, bias prefetch ----
        with ExitStack() as fes:
            fo_ps = fes.enter_context(
                tc.tile_pool(name="fo_ps", bufs=8, space="PSUM")
            )
            GB = 4
            NGRP = c.NV // GB
            bob_t = {}

            def load_bias(g):
                t = bobp.tile([128, GB * c.NF], BF16, name=f"bobg{g}", tag="bob")
                nc.scalar.dma_start(
                    out=t[:],
                    in_=bob[:, g * GB * c.NF : (g + 1) * GB * c.NF].to_broadcast(
                        [128, GB * c.NF]
                    ),
                )
                bob_t[g] = t

            for g in range(min(3, NGRP)):
                load_bias(g)

            for n in range(c.NV):
                wo_sb = wop.tile([128, c.KH, c.NF], BF16)
                if n % 2 == 0 or n < 7:
                    nc.sync.dma_start(out=wo_sb[:], in_=wot[n])
                else:
                    nc.gpsimd.dma_start(out=wo_sb[:], in_=wot[n])
                g = n // GB
                if n % GB == 0 and g + 3 < NGRP:
                    load_bias(g + 3)
                bob_n = bob_t[g][:, (n % GB) * c.NF : (n % GB + 1) * c.NF]
                for m in range(c.TM):
                    ps = fo_ps.tile([128, c.NF], F32)
                    for k in range(c.KH):
                        nc.tensor.matmul(
                            ps[:],
                            z[:, k, bass.ts(m, 128)],
                            wo_sb[:, k, :],
                            start=(k == 0),
                            stop=(k == c.KH - 1),
                        )
                    ot = outp.tile([128, c.NF], BF16)
                    nc.vector.tensor_add(ot[:], ps[:], bob_n)
                    nc.scalar.dma_start(
                        out=logits[bass.ts(m, 128), bass.ts(n, c.NF)], in_=ot[:]
                    )

    nc.compile()
    return nc


_CACHE = {}


def _get_nc_c(cfg):
    key = ("d",) + tuple(sorted(vars(cfg).items()))
    if key not in _CACHE:
        _CACHE[key] = build_nc_d(cfg)
    return _CACHE[key]


def run(inputs, trace=False, tmpdir=None):
    cfg = CfgC()
    nc = _get_nc_c(cfg)

    tokens = np.asarray(inputs["tokens"])
    B = tokens.shape[0]
    NQ = N_CORES // B
    E = np.asarray(inputs["E"], np.float32)
    e32c = np.ascontiguousarray(E)
    ebf16 = E.astype(BF)
    Wo = np.asarray(inputs["Wo"], np.float64)
    ln_w = np.asarray(inputs["ln_w"], np.float64)
    ln_b = np.asarray(inputs["ln_b"], np.float64)
    bo = np.asarray(inputs["bo"], np.float64)
    Wo_s = ln_w[:, None] * Wo / 10.0
    bo_s = (ln_b @ Wo + bo) / 10.0
    mlp = (inputs["Wc"], inputs["bc"], inputs["Wh"], inputs["bh"],
           inputs["Wm"], inputs["bm"])

    in_maps = []
    cache = {}
    for i in range(N_CORES):
        b, q = divmod(i, NQ)
        if (b, q) not in cache:
            cache[(b, q)] = prep_core_inputs_c(
                cfg, tokens[b], e32c, ebf16, Wo_s, bo_s, *mlp, q
            )
        in_maps.append(cache[(b, q)])

    res = run_bass_kernel_spmd(
        nc, in_maps, list(range(N_CORES)), trace=trace, tmpdir=tmpdir
    )
    out = np.empty((B, cfg.T, cfg.V), np.float32)
    for i in range(N_CORES):
        b, q = divmod(i, NQ)
        out[b, q * cfg.TL : (q + 1) * cfg.TL, :] = np.asarray(
            res.results[i]["logits"]
        ).astype(np.float32)
    return out, res


def kernel(**inputs) -> np.ndarray:
    out, _ = run(inputs, trace=False)
    return out

